# revision 12
# baseline (speedup 1.0000x reference)
"""Trainium2 Bass kernel: multi-head attention with 1x1-conv K/V projections,
per-head GhostBatchNorm (eval-mode affine), key+query masking, softmax.

Sharding: pure data parallelism over the batch axis (16 batches -> 8 cores),
with size-aware pairing: batches are sorted by unmasked count and each core
gets one small batch (compact length <= SPAD0, normally 512) and one large
batch (<= SPAD1, normally 544).  SPAD0 = 512 means the whole small batch runs
with single 512-wide PSUM pieces (no bank-split tail matmuls at all).

Host-side mask compaction (as v1): masked positions are dropped per batch,
kept positions gathered into compact arrays; a per-position valid flag
excludes pad columns from the softmax numerator/denominator via the v_pv
65th-column trick.

v2 changes vs the 278us/221us baseline:
  - fp16 for the whole q/k side (q, k_in, k_w, k_sb, scores matmul),
    bf16 for the v side (v_in, v_w, v_pv, E, PV matmul) and the output.
    Empirical pipeline sim: 0.67% max rel err (tolerance 2e-2).  Halves
    input DMA.  PE streams 1 row/cycle for both dtypes, but fp16/bf16
    allow the ldweights-reuse hack below and throttle less than fp32.
  - k_b dropped exactly: a key-bias shifts every score of a query column
    equally -> softmax invariant.  v_b folded exactly via the denominator:
    sum_s w (v+b) / sum_s w = PV/den + b, so v_pv = (v + b) * maskflag.
  - PSUM-piece tail matmuls (q columns 512..SPAD) set InstMatmult.ldweights
    = False and reuse the weights loaded by the 512-piece (verified on HW):
    small-N matmuls are otherwise paced by the ~225ns weight load.
  - Softmax reciprocal on DVE (reciprocal_approx_fast, ~51 ULP) instead of
    ACT Ln+Exp: kills the 42us of ACT_TABLE_LOAD thrash the tile scheduler
    caused by interleaving Ln/Exp with the main Exp stream.
  - Scores/exp/PV software-pipelined per head: PE order S(0) S(1) P(0)
    S(2) P(1) ... so exp(i) on ACT hides under scores(i+1).
  - Final normalize multiply on GpSimd (idle otherwise); o_raw PSUM->SBUF
    copy on ACT (Copy func, same Exp table, no swap).
  - 1/denominator partition-broadcast still via the DRAM-bounce DMA trick.
"""

import numpy as np

BS, DA, SL, H = 16, 512, 1024, 8
N_CORES = 8
B = BS // N_CORES  # batches per core (2)
P = 128
NT = DA // P       # channel tiles (4)
DH = DA // H       # head dim (64)

_CACHE: dict = {}


def _bucket(n: int) -> int:
    return max(128, -(-n // 32) * 32)


def _chunks(spad: int):
    cs = [(i * P, P) for i in range(spad // P)]
    if spad % P:
        cs.append((spad // P * P, spad % P))
    return cs


def _pieces(spad: int):
    if spad <= 512:
        return [(0, spad)]
    return [(0, 512), (512, spad - 512)]


def build_nc(spads):
    from contextlib import ExitStack

    import concourse.bass as bass
    import concourse.tile as tile
    from concourse import bacc, mybir

    f32 = mybir.dt.float32
    f16 = mybir.dt.float16
    bf16 = mybir.dt.bfloat16
    Alu = mybir.AluOpType
    Act = mybir.ActivationFunctionType

    nc = bacc.Bacc("TRN2", target_bir_lowering=False, debug=False)

    q_d, kin_d, vin_d, mf_d, out_d = [], [], [], [], []
    for b, spad in enumerate(spads):
        nch = len(_chunks(spad))
        q_d.append(nc.dram_tensor(f"q{b}", [DA, spad], f16, kind="ExternalInput"))
        kin_d.append(nc.dram_tensor(f"kin{b}", [DA, spad], f16, kind="ExternalInput"))
        vin_d.append(nc.dram_tensor(f"vin{b}", [DA, spad], bf16, kind="ExternalInput"))
        mf_d.append(nc.dram_tensor(f"mf{b}", [nch * P], f32, kind="ExternalInput"))
        out_d.append(nc.dram_tensor(f"out{b}", [DA, spad], bf16, kind="ExternalOutput"))
    kwT_d = nc.dram_tensor("kwT", [DA, DA], f16, kind="ExternalInput")
    vwT_d = nc.dram_tensor("vwT", [DA, DA], bf16, kind="ExternalInput")
    vb_d = nc.dram_tensor("vb", [1, DA], f32, kind="ExternalInput")

    with tile.TileContext(nc) as tc:
        with ExitStack() as ctx:
            consts = ctx.enter_context(tc.tile_pool(name="consts", bufs=1))
            inpool = ctx.enter_context(tc.tile_pool(name="inpool", bufs=2))
            kspool = ctx.enter_context(tc.tile_pool(name="kspool", bufs=2))
            vpvpool = ctx.enter_context(tc.tile_pool(name="vpvpool", bufs=2))
            vtpool = ctx.enter_context(tc.tile_pool(name="vtpool", bufs=2))
            epool = ctx.enter_context(tc.tile_pool(name="epool", bufs=3))
            scrpool = ctx.enter_context(tc.tile_pool(name="scrpool", bufs=3))
            bcpool = ctx.enter_context(tc.tile_pool(name="bcpool", bufs=3))
            opool = ctx.enter_context(tc.tile_pool(name="opool", bufs=3))
            psc = ctx.enter_context(tc.tile_pool(name="psc", bufs=2, space="PSUM"))
            ppv = ctx.enter_context(tc.tile_pool(name="ppv", bufs=2, space="PSUM"))

            # ---- constants + batch-0 input loads, ordered so the first
            # Kproj matmul can start as early as possible: kwT, then kin0
            # split per-ci chunk (the t=0/ci=0 matmul needs just the first
            # chunk), then q0/vin0/mf0, and only then vwT/vb (first needed
            # by Vproj, ~20us in).
            kwT_sb = consts.tile([P, NT, DA], f16)  # [p, ci, o]; c = ci*128+p
            nc.sync.dma_start(
                out=kwT_sb[:], in_=kwT_d.ap().rearrange("(ci p) o -> p ci o", p=P)
            )

            def issue_loads(b, spad):
                ncs = len(_chunks(spad))
                kin_sb = inpool.tile([P, NT, spad], f16, tag="kin", name=f"kin_{b}")
                for ci in range(NT):
                    nc.sync.dma_start(
                        out=kin_sb[:, ci, :],
                        in_=kin_d[b].ap()[ci * P : (ci + 1) * P, :],
                    )
                q_sb = inpool.tile([P, NT, spad], f16, tag="q", name=f"q_{b}")
                nc.sync.dma_start(
                    out=q_sb[:], in_=q_d[b].ap().rearrange("(t p) s -> p t s", p=P)
                )
                vin_sb = inpool.tile([P, NT, spad], bf16, tag="vin", name=f"vin_{b}")
                nc.sync.dma_start(
                    out=vin_sb[:], in_=vin_d[b].ap().rearrange("(t p) s -> p t s", p=P)
                )
                maskf = inpool.tile([P, ncs], f32, tag="mf", name=f"mf_{b}")
                nc.sync.dma_start(
                    out=maskf[:], in_=mf_d[b].ap().rearrange("(i p) -> p i", p=P)
                )
                return kin_sb, q_sb, vin_sb, maskf

            loads = {0: issue_loads(0, spads[0])}

            vwT_sb = consts.tile([P, NT, DA], bf16)
            nc.sync.dma_start(
                out=vwT_sb[:], in_=vwT_d.ap().rearrange("(ci p) o -> p ci o", p=P)
            )
            vb_bc = consts.tile([P, DA], f32)  # v_b broadcast to all partitions
            vb_row = vb_d.ap()[0]
            nc.sync.dma_start(
                out=vb_bc[:],
                in_=bass.AP(
                    tensor=vb_row.tensor,
                    offset=vb_row.offset,
                    ap=[[0, P]] + list(vb_row.ap),
                ),
            )
            ones8 = consts.tile([P, H], f32)
            nc.vector.memset(ones8[:], 1.0)
            negC = consts.tile([P, 1], f32)
            nc.vector.memset(negC[:], -45.0)

            for b, spad in enumerate(spads):
                cs = _chunks(spad)
                qps = _pieces(spad)
                ncs = len(cs)

                kin_sb, q_sb, vin_sb, maskf = loads.pop(b)
                if b + 1 < len(spads):
                    loads[b + 1] = issue_loads(b + 1, spads[b + 1])

                # ---- K projection (no bias; softmax shift-invariant) ----
                k_sb = kspool.tile([P, NT, spad], f16, tag="k", name=f"k_{b}")
                for t in range(NT):
                    kp = psc.tile([P, spad], f32, tag="sc", name=f"kp{t}")
                    for ci in range(NT):
                        lhsT = kwT_sb[:, ci, t * P : (t + 1) * P]
                        for qo, nq in qps:
                            nc.tensor.matmul(
                                kp[:, qo : qo + nq],
                                lhsT,
                                kin_sb[:, ci, qo : qo + nq],
                                start=(ci == 0),
                                stop=(ci == NT - 1),
                            )
                    nc.vector.tensor_copy(k_sb[:, t, :], kp[:, :])

                # ---- V projection (transposed) + v_pv assembly ----
                v_pv = vpvpool.tile([P, ncs, H, DH + 1], bf16, tag="vpv", name=f"vpv_{b}")
                for i, (s0, sc) in enumerate(cs):
                    vp = psc.tile([P, spad], f32, tag="sc", name=f"vp{i}")
                    for ci in range(NT):
                        nc.tensor.matmul(
                            vp[0:sc, 0:DA],
                            vin_sb[:, ci, s0 : s0 + sc],
                            vwT_sb[:, ci, :],
                            start=(ci == 0),
                            stop=(ci == NT - 1),
                        )
                    vt = vtpool.tile([P, DA], bf16, tag="vt", name=f"vt_{b}_{i}")
                    nc.vector.tensor_tensor(
                        vt[0:sc, :], vp[0:sc, 0:DA], vb_bc[0:sc, :], op=Alu.add
                    )
                    nc.vector.tensor_scalar_mul(
                        v_pv[0:sc, i, :, 0:DH],
                        vt[0:sc].rearrange("p (h d) -> p h d", h=H),
                        maskf[0:sc, i : i + 1],
                    )
                    nc.vector.tensor_scalar_mul(
                        v_pv[0:sc, i, :, DH], ones8[0:sc, :], maskf[0:sc, i : i + 1]
                    )

                # ---- attention, one head at a time, chunk-pipelined ----
                for h in range(H):
                    t, pb = h // 2, (h % 2) * DH
                    scs_tiles = {}

                    def emit_scores(i):
                        s0, sc = cs[i]
                        scs = psc.tile([P, spad], f32, tag="sc", name=f"sc{h}_{i}")
                        lhsT = k_sb[pb : pb + DH, t, s0 : s0 + sc]
                        for qo, nq in qps:
                            nc.tensor.matmul(
                                scs[0:sc, qo : qo + nq],
                                lhsT,
                                q_sb[pb : pb + DH, t, qo : qo + nq],
                                start=True,
                                stop=True,
                            )
                        scs_tiles[i] = scs

                    emit_scores(0)
                    pv = ppv.tile([DH + 1, spad], f32, tag="pv", name=f"pv{h}")
                    for i, (s0, sc) in enumerate(cs):
                        e_sb = epool.tile([P, spad], bf16, tag="e", name=f"e{h}_{i}")
                        # -45 shift keeps exp in fp32/bf16 range; softmax is
                        # shift-invariant.
                        nc.scalar.activation(
                            e_sb[0:sc, :],
                            scs_tiles.pop(i)[0:sc, :],
                            Act.Exp,
                            bias=negC[0:sc, 0:1],
                        )
                        if i + 1 < ncs:
                            emit_scores(i + 1)
                        lhsT = v_pv[0:sc, i, h, :]
                        for qo, nq in qps:
                            nc.tensor.matmul(
                                pv[0 : DH + 1, qo : qo + nq],
                                lhsT,
                                e_sb[0:sc, qo : qo + nq],
                                start=(i == 0),
                                stop=(i == ncs - 1),
                            )

                    # ---- epilogue: normalize by the accumulated denominator ----
                    # reciprocal_approx_fast is a custom-DVE op whose reads and
                    # writes are INVISIBLE to the tile dependency tracker: it is
                    # sandwiched between normal vector-engine ops (same queue ->
                    # program order) that carry the real dependencies.  Same for
                    # partition_broadcast on the gpsimd queue: a normal copy in
                    # front carries the cross-engine dependency, and the
                    # consumer (mult) sits behind it on the same queue.
                    o_raw = opool.tile([DH, spad], f32, tag="oraw", name=f"or{h}")
                    nc.scalar.activation(o_raw[:, :], pv[0:DH, :], Act.Copy)
                    den = scrpool.tile([1, spad], f32, tag="den", name=f"den{h}")
                    nc.vector.tensor_copy(den[0:1, :], pv[DH : DH + 1, :])
                    rec = scrpool.tile([1, spad], f32, tag="rec", name=f"rec{h}")
                    nc.vector.reciprocal_approx_fast(out=rec[0:1, :], in_=den[0:1, :])
                    rec2 = scrpool.tile([1, spad], f32, tag="rec2", name=f"rec2{h}")
                    nc.vector.tensor_copy(rec2[0:1, :], rec[0:1, :])
                    rec3 = scrpool.tile([1, spad], f32, tag="rec3", name=f"rec3{h}")
                    nc.gpsimd.tensor_copy(rec3[0:1, :], rec2[0:1, :])
                    bc = bcpool.tile([DH, spad], f32, tag="bc", name=f"bc{h}")
                    nc.gpsimd.partition_broadcast(bc[:, :], rec3[0:1, :])
                    o_sb = opool.tile([DH, spad], bf16, tag="osb", name=f"osb{h}")
                    nc.gpsimd.tensor_tensor(
                        o_sb[:, :], o_raw[:, :], bc[:, :], op=Alu.mult
                    )
                    nc.sync.dma_start(
                        out=out_d[b].ap()[h * DH : (h + 1) * DH, :], in_=o_sb[:, :]
                    )

    nc.compile()
    return nc


def _get_nc(spads):
    key = tuple(spads)
    if key not in _CACHE:
        _CACHE[key] = build_nc(key)
    return _CACHE[key]


def _prepare(inputs):
    """Mask compaction, GBN folding, size-aware batch pairing, sharding."""
    import ml_dtypes

    q = np.asarray(inputs["q"], dtype=np.float32)
    k_in = np.asarray(inputs["k_in"], dtype=np.float32)
    v_in = np.asarray(inputs["v_in"], dtype=np.float32)
    k_w = np.asarray(inputs["k_w"], dtype=np.float32)
    v_w = np.asarray(inputs["v_w"], dtype=np.float32)
    v_b = np.asarray(inputs["v_b"], dtype=np.float32)
    gamma = np.asarray(inputs["gbn_gamma"], dtype=np.float32)
    gs = np.asarray(inputs["gbn_s"], dtype=np.float32)
    mask = np.asarray(inputs["mask"]).reshape(BS, SL)

    # GBN affine: only gamma/sd matters (additive part and k_b are softmax
    # shift-invariant); fold the scale into q per head.
    a = (gamma / gs).astype(np.float32)
    q_scaled = (
        (q.reshape(BS, H, DH, SL) * a[None, :, None, None]).reshape(BS, DA, SL)
    ).astype(np.float32)

    keeps = [np.flatnonzero(mask[b] == 0) for b in range(BS)]
    counts = np.array([len(k) for k in keeps])
    order = np.argsort(counts, kind="stable")
    # large batches first: the kernel tail (last head's epilogue chain) then
    # belongs to the cheaper small batch.
    slots = [order[N_CORES:], order[:N_CORES]]
    spads = tuple(
        _bucket(int(counts[sl].max()) if len(sl) else 128) for sl in slots
    )

    k_wT = np.ascontiguousarray(k_w.T).astype(np.float16)
    v_wT = np.ascontiguousarray(v_w.T).astype(ml_dtypes.bfloat16)
    vb2 = v_b.reshape(1, DA).astype(np.float32)

    in_maps = [
        {"kwT": k_wT, "vwT": v_wT, "vb": vb2} for _ in range(N_CORES)
    ]
    assign = np.zeros((N_CORES, B), dtype=np.int64)
    for s, (sl, spad) in enumerate(zip(slots, spads)):
        nch = len(_chunks(spad))
        for c in range(N_CORES):
            gb = int(sl[c])
            assign[c, s] = gb
            kidx = keeps[gb]
            n = len(kidx)
            qc = np.zeros((DA, spad), np.float16)
            kc = np.zeros((DA, spad), np.float16)
            vc = np.zeros((DA, spad), ml_dtypes.bfloat16)
            mf = np.zeros((nch * P,), np.float32)
            qc[:, :n] = q_scaled[gb][:, kidx].astype(np.float16)
            kc[:, :n] = k_in[gb][:, kidx].astype(np.float16)
            vc[:, :n] = v_in[gb][:, kidx].astype(ml_dtypes.bfloat16)
            mf[:n] = 1.0
            in_maps[c][f"q{s}"] = qc
            in_maps[c][f"kin{s}"] = kc
            in_maps[c][f"vin{s}"] = vc
            in_maps[c][f"mf{s}"] = mf
    return in_maps, keeps, assign, spads


def _scatter(results, keeps, assign) -> np.ndarray:
    out = np.zeros((BS, DA, SL), np.float32)
    for c in range(N_CORES):
        for s in range(B):
            gb = int(assign[c, s])
            kidx = keeps[gb]
            oc = np.asarray(results[c][f"out{s}"]).astype(np.float32)
            out[gb][:, kidx] = oc[:, : len(kidx)]
    return out


def kernel(**inputs) -> np.ndarray:
    from concourse.bass_utils import run_bass_kernel_spmd

    in_maps, keeps, assign, spads = _prepare(inputs)
    nc = _get_nc(spads)
    res = run_bass_kernel_spmd(nc, in_maps, list(range(N_CORES)))
    return _scatter(res.results, keeps, assign)


# revision 14
# speedup vs baseline: 2.2088x; 2.2088x over previous
"""Trainium2 Bass kernel: multi-head attention with 1x1-conv K/V projections,
per-head GhostBatchNorm (eval-mode affine), key+query masking, softmax.

Sharding: pure data parallelism over the batch axis (16 batches -> 8 cores),
with size-aware pairing: batches are sorted by unmasked count and each core
gets one small batch (compact length <= SPAD0, normally 512) and one large
batch (<= SPAD1, normally 544).  SPAD0 = 512 means the whole small batch runs
with single 512-wide PSUM pieces (no bank-split tail matmuls at all).

Host-side mask compaction (as v1): masked positions are dropped per batch,
kept positions gathered into compact arrays; a per-position valid flag
excludes pad columns from the softmax numerator/denominator via the v_pv
65th-column trick.

v2 changes vs the 278us/221us baseline:
  - fp16 for the whole q/k side (q, k_in, k_w, k_sb, scores matmul),
    bf16 for the v side (v_in, v_w, v_pv, E, PV matmul) and the output.
    Empirical pipeline sim: 0.67% max rel err (tolerance 2e-2).  Halves
    input DMA.  PE streams 1 row/cycle for both dtypes, but fp16/bf16
    allow the ldweights-reuse hack below and throttle less than fp32.
  - k_b dropped exactly: a key-bias shifts every score of a query column
    equally -> softmax invariant.  v_b folded exactly via the denominator:
    sum_s w (v+b) / sum_s w = PV/den + b, so v_pv = (v + b) * maskflag.
  - PSUM-piece tail matmuls (q columns 512..SPAD) set InstMatmult.ldweights
    = False and reuse the weights loaded by the 512-piece (verified on HW):
    small-N matmuls are otherwise paced by the ~225ns weight load.
  - Softmax reciprocal on DVE (reciprocal_approx_fast, ~51 ULP) instead of
    ACT Ln+Exp: kills the 42us of ACT_TABLE_LOAD thrash the tile scheduler
    caused by interleaving Ln/Exp with the main Exp stream.
  - Scores/exp/PV software-pipelined per head: PE order S(0) S(1) P(0)
    S(2) P(1) ... so exp(i) on ACT hides under scores(i+1).
  - Final normalize multiply on GpSimd (idle otherwise); o_raw PSUM->SBUF
    copy on ACT (Copy func, same Exp table, no swap).
  - 1/denominator partition-broadcast still via the DRAM-bounce DMA trick.
"""

import numpy as np

BS, DA, SL, H = 16, 512, 1024, 8
N_CORES = 8
B = BS // N_CORES  # batches per core (2)
P = 128
NT = DA // P       # channel tiles (4)
DH = DA // H       # head dim (64)

_CACHE: dict = {}


def _bucket(n: int) -> int:
    return max(128, -(-n // 32) * 32)


def _chunks(spad: int):
    cs = [(i * P, P) for i in range(spad // P)]
    if spad % P:
        cs.append((spad // P * P, spad % P))
    return cs


def _pieces(spad: int):
    if spad <= 512:
        return [(0, spad)]
    return [(0, 512), (512, spad - 512)]


def build_nc(spads):
    from contextlib import ExitStack

    import concourse.bass as bass
    import concourse.tile as tile
    from concourse import bacc, mybir

    f32 = mybir.dt.float32
    f16 = mybir.dt.float16
    bf16 = mybir.dt.bfloat16
    Alu = mybir.AluOpType
    Act = mybir.ActivationFunctionType

    nc = bacc.Bacc("TRN2", target_bir_lowering=False, debug=False)

    q_d, kin_d, vin_d, mf_d, out_d = [], [], [], [], []
    for b, spad in enumerate(spads):
        nch = len(_chunks(spad))
        q_d.append(nc.dram_tensor(f"q{b}", [DA, spad], f16, kind="ExternalInput"))
        kin_d.append(nc.dram_tensor(f"kin{b}", [DA, spad], f16, kind="ExternalInput"))
        vin_d.append(nc.dram_tensor(f"vin{b}", [DA, spad], bf16, kind="ExternalInput"))
        mf_d.append(nc.dram_tensor(f"mf{b}", [nch * P], f32, kind="ExternalInput"))
        out_d.append(nc.dram_tensor(f"out{b}", [DA, spad], bf16, kind="ExternalOutput"))
    scr_d = [
        nc.dram_tensor(f"scr{b}", [H, spad], f32) for b, spad in enumerate(spads)
    ]
    kwT_d = nc.dram_tensor("kwT", [DA, DA], f16, kind="ExternalInput")
    vwT_d = nc.dram_tensor("vwT", [DA, DA], bf16, kind="ExternalInput")
    vb_d = nc.dram_tensor("vb", [1, DA], f32, kind="ExternalInput")

    with tile.TileContext(nc) as tc:
        with ExitStack() as ctx:
            consts = ctx.enter_context(tc.tile_pool(name="consts", bufs=1))
            inpool = ctx.enter_context(tc.tile_pool(name="inpool", bufs=2))
            kspool = ctx.enter_context(tc.tile_pool(name="kspool", bufs=2))
            vpvpool = ctx.enter_context(tc.tile_pool(name="vpvpool", bufs=2))
            vtpool = ctx.enter_context(tc.tile_pool(name="vtpool", bufs=2))
            epool = ctx.enter_context(tc.tile_pool(name="epool", bufs=3))
            scrpool = ctx.enter_context(tc.tile_pool(name="scrpool", bufs=3))
            bcpool = ctx.enter_context(tc.tile_pool(name="bcpool", bufs=3))
            opool = ctx.enter_context(tc.tile_pool(name="opool", bufs=3))
            psc = ctx.enter_context(tc.tile_pool(name="psc", bufs=2, space="PSUM"))
            ppv = ctx.enter_context(tc.tile_pool(name="ppv", bufs=2, space="PSUM"))

            # ---- constants + batch-0 input loads, ordered so the first
            # Kproj matmul can start as early as possible: kwT, then kin0
            # split per-ci chunk (the t=0/ci=0 matmul needs just the first
            # chunk), then q0/vin0/mf0, and only then vwT/vb (first needed
            # by Vproj, ~20us in).
            kwT_sb = consts.tile([P, NT, DA], f16)  # [p, ci, o]; c = ci*128+p
            nc.sync.dma_start(
                out=kwT_sb[:], in_=kwT_d.ap().rearrange("(ci p) o -> p ci o", p=P)
            )

            def issue_loads(b, spad):
                ncs = len(_chunks(spad))
                kin_sb = inpool.tile([P, NT, spad], f16, tag="kin", name=f"kin_{b}")
                for ci in range(NT):
                    nc.sync.dma_start(
                        out=kin_sb[:, ci, :],
                        in_=kin_d[b].ap()[ci * P : (ci + 1) * P, :],
                    )
                q_sb = inpool.tile([P, NT, spad], f16, tag="q", name=f"q_{b}")
                nc.sync.dma_start(
                    out=q_sb[:], in_=q_d[b].ap().rearrange("(t p) s -> p t s", p=P)
                )
                vin_sb = inpool.tile([P, NT, spad], bf16, tag="vin", name=f"vin_{b}")
                nc.sync.dma_start(
                    out=vin_sb[:], in_=vin_d[b].ap().rearrange("(t p) s -> p t s", p=P)
                )
                maskf = inpool.tile([P, ncs], f32, tag="mf", name=f"mf_{b}")
                nc.sync.dma_start(
                    out=maskf[:], in_=mf_d[b].ap().rearrange("(i p) -> p i", p=P)
                )
                return kin_sb, q_sb, vin_sb, maskf

            loads = {0: issue_loads(0, spads[0])}

            vwT_sb = consts.tile([P, NT, DA], bf16)
            nc.sync.dma_start(
                out=vwT_sb[:], in_=vwT_d.ap().rearrange("(ci p) o -> p ci o", p=P)
            )
            vb_bc = consts.tile([P, DA], f32)  # v_b broadcast to all partitions
            vb_row = vb_d.ap()[0]
            nc.sync.dma_start(
                out=vb_bc[:],
                in_=bass.AP(
                    tensor=vb_row.tensor,
                    offset=vb_row.offset,
                    ap=[[0, P]] + list(vb_row.ap),
                ),
            )
            ones8 = consts.tile([P, H], f32)
            nc.vector.memset(ones8[:], 1.0)
            negC = consts.tile([P, 1], f32)
            nc.vector.memset(negC[:], -45.0)

            for b, spad in enumerate(spads):
                cs = _chunks(spad)
                qps = _pieces(spad)
                ncs = len(cs)

                kin_sb, q_sb, vin_sb, maskf = loads.pop(b)
                if b + 1 < len(spads):
                    loads[b + 1] = issue_loads(b + 1, spads[b + 1])

                # ---- K projection (no bias; softmax shift-invariant) ----
                k_sb = kspool.tile([P, NT, spad], f16, tag="k", name=f"k_{b}")
                for t in range(NT):
                    kp = psc.tile([P, spad], f32, tag="sc", name=f"kp{t}")
                    for ci in range(NT):
                        lhsT = kwT_sb[:, ci, t * P : (t + 1) * P]
                        for qo, nq in qps:
                            nc.tensor.matmul(
                                kp[:, qo : qo + nq],
                                lhsT,
                                kin_sb[:, ci, qo : qo + nq],
                                start=(ci == 0),
                                stop=(ci == NT - 1),
                            )
                    nc.vector.tensor_copy(k_sb[:, t, :], kp[:, :])

                # ---- V projection (transposed) + v_pv assembly ----
                v_pv = vpvpool.tile([P, ncs, H, DH + 1], bf16, tag="vpv", name=f"vpv_{b}")
                for i, (s0, sc) in enumerate(cs):
                    vp = psc.tile([P, spad], f32, tag="sc", name=f"vp{i}")
                    for ci in range(NT):
                        nc.tensor.matmul(
                            vp[0:sc, 0:DA],
                            vin_sb[:, ci, s0 : s0 + sc],
                            vwT_sb[:, ci, :],
                            start=(ci == 0),
                            stop=(ci == NT - 1),
                        )
                    vt = vtpool.tile([P, DA], bf16, tag="vt", name=f"vt_{b}_{i}")
                    nc.vector.tensor_tensor(
                        vt[0:sc, :], vp[0:sc, 0:DA], vb_bc[0:sc, :], op=Alu.add
                    )
                    nc.vector.tensor_scalar_mul(
                        v_pv[0:sc, i, :, 0:DH],
                        vt[0:sc].rearrange("p (h d) -> p h d", h=H),
                        maskf[0:sc, i : i + 1],
                    )
                    nc.vector.tensor_scalar_mul(
                        v_pv[0:sc, i, :, DH], ones8[0:sc, :], maskf[0:sc, i : i + 1]
                    )

                # ---- attention, one head at a time, chunk-pipelined ----
                for h in range(H):
                    t, pb = h // 2, (h % 2) * DH
                    scs_tiles = {}

                    def emit_scores(i):
                        s0, sc = cs[i]
                        scs = psc.tile([P, spad], f32, tag="sc", name=f"sc{h}_{i}")
                        lhsT = k_sb[pb : pb + DH, t, s0 : s0 + sc]
                        for qo, nq in qps:
                            nc.tensor.matmul(
                                scs[0:sc, qo : qo + nq],
                                lhsT,
                                q_sb[pb : pb + DH, t, qo : qo + nq],
                                start=True,
                                stop=True,
                            )
                        scs_tiles[i] = scs

                    emit_scores(0)
                    pv = ppv.tile([DH + 1, spad], f32, tag="pv", name=f"pv{h}")
                    for i, (s0, sc) in enumerate(cs):
                        e_sb = epool.tile([P, spad], bf16, tag="e", name=f"e{h}_{i}")
                        # -45 shift keeps exp in fp32/bf16 range; softmax is
                        # shift-invariant.
                        nc.scalar.activation(
                            e_sb[0:sc, :],
                            scs_tiles.pop(i)[0:sc, :],
                            Act.Exp,
                            bias=negC[0:sc, 0:1],
                        )
                        if i + 1 < ncs:
                            emit_scores(i + 1)
                        lhsT = v_pv[0:sc, i, h, :]
                        for qo, nq in qps:
                            nc.tensor.matmul(
                                pv[0 : DH + 1, qo : qo + nq],
                                lhsT,
                                e_sb[0:sc, qo : qo + nq],
                                start=(i == 0),
                                stop=(i == ncs - 1),
                            )

                    # ---- epilogue: normalize by the accumulated denominator ----
                    # reciprocal_approx_fast is a custom-DVE op whose reads and
                    # writes are INVISIBLE to the tile dependency tracker: it is
                    # sandwiched between normal vector-engine ops (same queue ->
                    # program order) that carry the real dependencies.  Same for
                    # partition_broadcast on the gpsimd queue: a normal copy in
                    # front carries the cross-engine dependency, and the
                    # consumer (mult) sits behind it on the same queue.
                    o_raw = opool.tile([DH, spad], f32, tag="oraw", name=f"or{h}")
                    nc.scalar.activation(o_raw[:, :], pv[0:DH, :], Act.Copy)
                    den = scrpool.tile([1, spad], f32, tag="den", name=f"den{h}")
                    nc.vector.tensor_copy(den[0:1, :], pv[DH : DH + 1, :])
                    rec = scrpool.tile([1, spad], f32, tag="rec", name=f"rec{h}")
                    nc.vector.reciprocal_approx_fast(out=rec[0:1, :], in_=den[0:1, :])
                    rec2 = scrpool.tile([1, spad], f32, tag="rec2", name=f"rec2{h}")
                    nc.vector.tensor_copy(rec2[0:1, :], rec[0:1, :])
                    # partition-broadcast via DRAM bounce: a 0-stride partition
                    # AP reads the row 64x.  (gpsimd partition_broadcast is ~5us
                    # of serial engine time per head and back-pressures the
                    # whole pipeline; the DMA path is much faster.)
                    row = scr_d[b].ap()[h]
                    nc.sync.dma_start(out=row, in_=rec2[0:1, :])
                    bc = bcpool.tile([DH, spad], f32, tag="bc", name=f"bc{h}")
                    nc.sync.dma_start(
                        out=bc[:, :],
                        in_=bass.AP(
                            tensor=row.tensor,
                            offset=row.offset,
                            ap=[[0, DH]] + list(row.ap),
                        ),
                    )
                    o_sb = opool.tile([DH, spad], bf16, tag="osb", name=f"osb{h}")
                    nc.gpsimd.tensor_tensor(
                        o_sb[:, :], o_raw[:, :], bc[:, :], op=Alu.mult
                    )
                    nc.sync.dma_start(
                        out=out_d[b].ap()[h * DH : (h + 1) * DH, :], in_=o_sb[:, :]
                    )

    nc.compile()
    return nc


def _get_nc(spads):
    key = tuple(spads)
    if key not in _CACHE:
        _CACHE[key] = build_nc(key)
    return _CACHE[key]


def _prepare(inputs):
    """Mask compaction, GBN folding, size-aware batch pairing, sharding."""
    import ml_dtypes

    q = np.asarray(inputs["q"], dtype=np.float32)
    k_in = np.asarray(inputs["k_in"], dtype=np.float32)
    v_in = np.asarray(inputs["v_in"], dtype=np.float32)
    k_w = np.asarray(inputs["k_w"], dtype=np.float32)
    v_w = np.asarray(inputs["v_w"], dtype=np.float32)
    v_b = np.asarray(inputs["v_b"], dtype=np.float32)
    gamma = np.asarray(inputs["gbn_gamma"], dtype=np.float32)
    gs = np.asarray(inputs["gbn_s"], dtype=np.float32)
    mask = np.asarray(inputs["mask"]).reshape(BS, SL)

    # GBN affine: only gamma/sd matters (additive part and k_b are softmax
    # shift-invariant); fold the scale into q per head.
    a = (gamma / gs).astype(np.float32)
    q_scaled = (
        (q.reshape(BS, H, DH, SL) * a[None, :, None, None]).reshape(BS, DA, SL)
    ).astype(np.float32)

    keeps = [np.flatnonzero(mask[b] == 0) for b in range(BS)]
    counts = np.array([len(k) for k in keeps])
    order = np.argsort(counts, kind="stable")
    # large batches first: the kernel tail (last head's epilogue chain) then
    # belongs to the cheaper small batch.
    slots = [order[N_CORES:], order[:N_CORES]]
    spads = tuple(
        _bucket(int(counts[sl].max()) if len(sl) else 128) for sl in slots
    )

    k_wT = np.ascontiguousarray(k_w.T).astype(np.float16)
    v_wT = np.ascontiguousarray(v_w.T).astype(ml_dtypes.bfloat16)
    vb2 = v_b.reshape(1, DA).astype(np.float32)

    in_maps = [
        {"kwT": k_wT, "vwT": v_wT, "vb": vb2} for _ in range(N_CORES)
    ]
    assign = np.zeros((N_CORES, B), dtype=np.int64)
    for s, (sl, spad) in enumerate(zip(slots, spads)):
        nch = len(_chunks(spad))
        for c in range(N_CORES):
            gb = int(sl[c])
            assign[c, s] = gb
            kidx = keeps[gb]
            n = len(kidx)
            qc = np.zeros((DA, spad), np.float16)
            kc = np.zeros((DA, spad), np.float16)
            vc = np.zeros((DA, spad), ml_dtypes.bfloat16)
            mf = np.zeros((nch * P,), np.float32)
            qc[:, :n] = q_scaled[gb][:, kidx].astype(np.float16)
            kc[:, :n] = k_in[gb][:, kidx].astype(np.float16)
            vc[:, :n] = v_in[gb][:, kidx].astype(ml_dtypes.bfloat16)
            mf[:n] = 1.0
            in_maps[c][f"q{s}"] = qc
            in_maps[c][f"kin{s}"] = kc
            in_maps[c][f"vin{s}"] = vc
            in_maps[c][f"mf{s}"] = mf
    return in_maps, keeps, assign, spads


def _scatter(results, keeps, assign) -> np.ndarray:
    out = np.zeros((BS, DA, SL), np.float32)
    for c in range(N_CORES):
        for s in range(B):
            gb = int(assign[c, s])
            kidx = keeps[gb]
            oc = np.asarray(results[c][f"out{s}"]).astype(np.float32)
            out[gb][:, kidx] = oc[:, : len(kidx)]
    return out


def kernel(**inputs) -> np.ndarray:
    from concourse.bass_utils import run_bass_kernel_spmd

    in_maps, keeps, assign, spads = _prepare(inputs)
    nc = _get_nc(spads)
    res = run_bass_kernel_spmd(nc, in_maps, list(range(N_CORES)))
    return _scatter(res.results, keeps, assign)


# revision 16
# speedup vs baseline: 2.4004x; 1.0868x over previous
"""Trainium2 Bass kernel: multi-head attention with 1x1-conv K/V projections,
per-head GhostBatchNorm (eval-mode affine), key+query masking, softmax.

Sharding: pure data parallelism over the batch axis (16 batches -> 8 cores),
with size-aware pairing: batches are sorted by unmasked count and each core
gets one small batch (compact length <= SPAD0, normally 512) and one large
batch (<= SPAD1, normally 544).  SPAD0 = 512 means the whole small batch runs
with single 512-wide PSUM pieces (no bank-split tail matmuls at all).

Host-side mask compaction (as v1): masked positions are dropped per batch,
kept positions gathered into compact arrays; a per-position valid flag
excludes pad columns from the softmax numerator/denominator via the v_pv
65th-column trick.

Changes vs the 278us/221us baseline (measured 133-142us, clock-variance):
  - fp16 for the whole q/k side (q, k_in, k_w, k_sb, scores matmul),
    bf16 for the v side (v_in, v_w, v_pv, E, PV matmul) and the output.
    Empirical pipeline sim: 0.67% max rel err (tolerance 2e-2).  Halves
    input DMA; fp16/bf16 weight loads (~114ns) hide under 512-row streams.
  - k_b dropped exactly: a key-bias shifts every score of a query column
    equally -> softmax invariant.  v_b folded exactly via the denominator:
    sum_s w (v+b) / sum_s w = PV/den + b, so v_pv = (v + b) * maskflag.
  - Softmax reciprocal on DVE (reciprocal_approx_fast, ~51 ULP) instead of
    ACT Ln+Exp: kills the 42us of ACT_TABLE_LOAD thrash the tile scheduler
    caused by interleaving Ln/Exp with the main Exp stream.  The custom-DVE
    op's reads/writes are invisible to the tile dependency tracker, so it
    is sandwiched between normal vector-engine ops (same queue = program
    order): den copy in front, rec2 copy behind.
  - Scores/exp/PV software-pipelined per head: PE order S(0) S(1) P(0)
    S(2) P(1) ... so exp(i) on ACT hides under scores(i+1).
  - Final normalize multiply on GpSimd (idle otherwise; it cannot read
    PSUM, hence the o_raw PSUM->SBUF ACT Copy, same Exp table, no swap).
  - 1/denominator partition-broadcast via the DRAM-bounce DMA trick
    (gpsimd partition_broadcast measured ~5us/head serial engine time and
    stalled the PE ~12us per head through pool back-pressure).
  - Input DMAs issued in first-use order, kwT/kin split per ci chunk, so
    the first Kproj matmul starts ~9us in instead of ~23us.
"""

import numpy as np

BS, DA, SL, H = 16, 512, 1024, 8
N_CORES = 8
B = BS // N_CORES  # batches per core (2)
P = 128
NT = DA // P       # channel tiles (4)
DH = DA // H       # head dim (64)

_CACHE: dict = {}


def _bucket(n: int) -> int:
    return max(128, -(-n // 32) * 32)


def _chunks(spad: int):
    cs = [(i * P, P) for i in range(spad // P)]
    if spad % P:
        cs.append((spad // P * P, spad % P))
    return cs


def _pieces(spad: int):
    if spad <= 512:
        return [(0, spad)]
    return [(0, 512), (512, spad - 512)]


def build_nc(spads):
    from contextlib import ExitStack

    import concourse.bass as bass
    import concourse.tile as tile
    from concourse import bacc, mybir

    f32 = mybir.dt.float32
    f16 = mybir.dt.float16
    bf16 = mybir.dt.bfloat16
    Alu = mybir.AluOpType
    Act = mybir.ActivationFunctionType

    nc = bacc.Bacc("TRN2", target_bir_lowering=False, debug=False)

    q_d, kin_d, vin_d, mf_d, out_d = [], [], [], [], []
    for b, spad in enumerate(spads):
        nch = len(_chunks(spad))
        q_d.append(nc.dram_tensor(f"q{b}", [DA, spad], f16, kind="ExternalInput"))
        kin_d.append(nc.dram_tensor(f"kin{b}", [DA, spad], f16, kind="ExternalInput"))
        vin_d.append(nc.dram_tensor(f"vin{b}", [DA, spad], bf16, kind="ExternalInput"))
        mf_d.append(nc.dram_tensor(f"mf{b}", [nch * P], f32, kind="ExternalInput"))
        out_d.append(nc.dram_tensor(f"out{b}", [DA, spad], bf16, kind="ExternalOutput"))
    scr_d = [
        nc.dram_tensor(f"scr{b}", [H, spad], f32) for b, spad in enumerate(spads)
    ]
    kwT_d = nc.dram_tensor("kwT", [DA, DA], f16, kind="ExternalInput")
    vwT_d = nc.dram_tensor("vwT", [DA, DA], bf16, kind="ExternalInput")
    vb_d = nc.dram_tensor("vb", [1, DA], f32, kind="ExternalInput")

    with tile.TileContext(nc) as tc:
        with ExitStack() as ctx:
            consts = ctx.enter_context(tc.tile_pool(name="consts", bufs=1))
            inpool = ctx.enter_context(tc.tile_pool(name="inpool", bufs=2))
            kspool = ctx.enter_context(tc.tile_pool(name="kspool", bufs=2))
            vpvpool = ctx.enter_context(tc.tile_pool(name="vpvpool", bufs=2))
            vtpool = ctx.enter_context(tc.tile_pool(name="vtpool", bufs=2))
            epool = ctx.enter_context(tc.tile_pool(name="epool", bufs=3))
            scrpool = ctx.enter_context(tc.tile_pool(name="scrpool", bufs=3))
            bcpool = ctx.enter_context(tc.tile_pool(name="bcpool", bufs=3))
            opool = ctx.enter_context(tc.tile_pool(name="opool", bufs=3))
            psc = ctx.enter_context(tc.tile_pool(name="psc", bufs=2, space="PSUM"))
            ppv = ctx.enter_context(tc.tile_pool(name="ppv", bufs=2, space="PSUM"))

            # ---- constants + batch-0 input loads, ordered so the first
            # Kproj matmul can start as early as possible: kwT, then kin0
            # split per-ci chunk (the t=0/ci=0 matmul needs just the first
            # chunk), then q0/vin0/mf0, and only then vwT/vb (first needed
            # by Vproj, ~20us in).
            kwT_sb = consts.tile([P, NT, DA], f16)  # [p, ci, o]; c = ci*128+p

            def issue_loads(b, spad):
                ncs = len(_chunks(spad))
                kin_sb = inpool.tile([P, NT, spad], f16, tag="kin", name=f"kin_{b}")
                for ci in range(NT):
                    if b == 0:
                        # interleave weight/input chunks in Kproj consumption
                        # order so the t=0/ci=0 matmul starts earliest
                        nc.sync.dma_start(
                            out=kwT_sb[:, ci, :],
                            in_=kwT_d.ap()[ci * P : (ci + 1) * P, :],
                        )
                    nc.sync.dma_start(
                        out=kin_sb[:, ci, :],
                        in_=kin_d[b].ap()[ci * P : (ci + 1) * P, :],
                    )
                q_sb = inpool.tile([P, NT, spad], f16, tag="q", name=f"q_{b}")
                nc.sync.dma_start(
                    out=q_sb[:], in_=q_d[b].ap().rearrange("(t p) s -> p t s", p=P)
                )
                vin_sb = inpool.tile([P, NT, spad], bf16, tag="vin", name=f"vin_{b}")
                nc.sync.dma_start(
                    out=vin_sb[:], in_=vin_d[b].ap().rearrange("(t p) s -> p t s", p=P)
                )
                maskf = inpool.tile([P, ncs], f32, tag="mf", name=f"mf_{b}")
                nc.sync.dma_start(
                    out=maskf[:], in_=mf_d[b].ap().rearrange("(i p) -> p i", p=P)
                )
                return kin_sb, q_sb, vin_sb, maskf

            loads = {0: issue_loads(0, spads[0])}

            vwT_sb = consts.tile([P, NT, DA], bf16)
            nc.sync.dma_start(
                out=vwT_sb[:], in_=vwT_d.ap().rearrange("(ci p) o -> p ci o", p=P)
            )
            vb_bc = consts.tile([P, DA], f32)  # v_b broadcast to all partitions
            vb_row = vb_d.ap()[0]
            nc.sync.dma_start(
                out=vb_bc[:],
                in_=bass.AP(
                    tensor=vb_row.tensor,
                    offset=vb_row.offset,
                    ap=[[0, P]] + list(vb_row.ap),
                ),
            )
            ones8 = consts.tile([P, H], f32)
            nc.vector.memset(ones8[:], 1.0)
            negC = consts.tile([P, 1], f32)
            nc.vector.memset(negC[:], -45.0)

            for b, spad in enumerate(spads):
                cs = _chunks(spad)
                qps = _pieces(spad)
                ncs = len(cs)

                kin_sb, q_sb, vin_sb, maskf = loads.pop(b)
                if b + 1 < len(spads):
                    loads[b + 1] = issue_loads(b + 1, spads[b + 1])

                # ---- K projection (no bias; softmax shift-invariant) ----
                k_sb = kspool.tile([P, NT, spad], f16, tag="k", name=f"k_{b}")
                for t in range(NT):
                    kp = psc.tile([P, spad], f32, tag="sc", name=f"kp{t}")
                    for ci in range(NT):
                        lhsT = kwT_sb[:, ci, t * P : (t + 1) * P]
                        for qo, nq in qps:
                            nc.tensor.matmul(
                                kp[:, qo : qo + nq],
                                lhsT,
                                kin_sb[:, ci, qo : qo + nq],
                                start=(ci == 0),
                                stop=(ci == NT - 1),
                            )
                    nc.vector.tensor_copy(k_sb[:, t, :], kp[:, :])

                # ---- V projection (transposed) + v_pv assembly ----
                v_pv = vpvpool.tile([P, ncs, H, DH + 1], bf16, tag="vpv", name=f"vpv_{b}")
                for i, (s0, sc) in enumerate(cs):
                    vp = psc.tile([P, spad], f32, tag="sc", name=f"vp{i}")
                    for ci in range(NT):
                        nc.tensor.matmul(
                            vp[0:sc, 0:DA],
                            vin_sb[:, ci, s0 : s0 + sc],
                            vwT_sb[:, ci, :],
                            start=(ci == 0),
                            stop=(ci == NT - 1),
                        )
                    vt = vtpool.tile([P, DA], bf16, tag="vt", name=f"vt_{b}_{i}")
                    nc.vector.tensor_tensor(
                        vt[0:sc, :], vp[0:sc, 0:DA], vb_bc[0:sc, :], op=Alu.add
                    )
                    nc.vector.tensor_scalar_mul(
                        v_pv[0:sc, i, :, 0:DH],
                        vt[0:sc].rearrange("p (h d) -> p h d", h=H),
                        maskf[0:sc, i : i + 1],
                    )
                    nc.vector.tensor_scalar_mul(
                        v_pv[0:sc, i, :, DH], ones8[0:sc, :], maskf[0:sc, i : i + 1]
                    )

                # ---- attention, one head at a time, chunk-pipelined ----
                for h in range(H):
                    t, pb = h // 2, (h % 2) * DH
                    scs_tiles = {}

                    def emit_scores(i):
                        s0, sc = cs[i]
                        scs = psc.tile([P, spad], f32, tag="sc", name=f"sc{h}_{i}")
                        lhsT = k_sb[pb : pb + DH, t, s0 : s0 + sc]
                        for qo, nq in qps:
                            nc.tensor.matmul(
                                scs[0:sc, qo : qo + nq],
                                lhsT,
                                q_sb[pb : pb + DH, t, qo : qo + nq],
                                start=True,
                                stop=True,
                            )
                        scs_tiles[i] = scs

                    emit_scores(0)
                    pv = ppv.tile([DH + 1, spad], f32, tag="pv", name=f"pv{h}")
                    for i, (s0, sc) in enumerate(cs):
                        e_sb = epool.tile([P, spad], bf16, tag="e", name=f"e{h}_{i}")
                        # -45 shift keeps exp in fp32/bf16 range; softmax is
                        # shift-invariant.
                        nc.scalar.activation(
                            e_sb[0:sc, :],
                            scs_tiles.pop(i)[0:sc, :],
                            Act.Exp,
                            bias=negC[0:sc, 0:1],
                        )
                        if i + 1 < ncs:
                            emit_scores(i + 1)
                        lhsT = v_pv[0:sc, i, h, :]
                        for qo, nq in qps:
                            nc.tensor.matmul(
                                pv[0 : DH + 1, qo : qo + nq],
                                lhsT,
                                e_sb[0:sc, qo : qo + nq],
                                start=(i == 0),
                                stop=(i == ncs - 1),
                            )

                    # ---- epilogue: normalize by the accumulated denominator ----
                    # reciprocal_approx_fast is a custom-DVE op whose reads and
                    # writes are INVISIBLE to the tile dependency tracker: it is
                    # sandwiched between normal vector-engine ops (same queue ->
                    # program order) that carry the real dependencies.  Same for
                    # partition_broadcast on the gpsimd queue: a normal copy in
                    # front carries the cross-engine dependency, and the
                    # consumer (mult) sits behind it on the same queue.
                    o_raw = opool.tile([DH, spad], f32, tag="oraw", name=f"or{h}")
                    nc.scalar.activation(o_raw[:, :], pv[0:DH, :], Act.Copy)
                    den = scrpool.tile([1, spad], f32, tag="den", name=f"den{h}")
                    nc.vector.tensor_copy(den[0:1, :], pv[DH : DH + 1, :])
                    rec = scrpool.tile([1, spad], f32, tag="rec", name=f"rec{h}")
                    nc.vector.reciprocal_approx_fast(out=rec[0:1, :], in_=den[0:1, :])
                    rec2 = scrpool.tile([1, spad], f32, tag="rec2", name=f"rec2{h}")
                    nc.vector.tensor_copy(rec2[0:1, :], rec[0:1, :])
                    # partition-broadcast via DRAM bounce: a 0-stride partition
                    # AP reads the row 64x.  (gpsimd partition_broadcast is ~5us
                    # of serial engine time per head and back-pressures the
                    # whole pipeline; the DMA path is much faster.)
                    row = scr_d[b].ap()[h]
                    nc.sync.dma_start(out=row, in_=rec2[0:1, :])
                    bc = bcpool.tile([DH, spad], f32, tag="bc", name=f"bc{h}")
                    nc.sync.dma_start(
                        out=bc[:, :],
                        in_=bass.AP(
                            tensor=row.tensor,
                            offset=row.offset,
                            ap=[[0, DH]] + list(row.ap),
                        ),
                    )
                    o_sb = opool.tile([DH, spad], bf16, tag="osb", name=f"osb{h}")
                    nc.gpsimd.tensor_tensor(
                        o_sb[:, :], o_raw[:, :], bc[:, :], op=Alu.mult
                    )
                    nc.sync.dma_start(
                        out=out_d[b].ap()[h * DH : (h + 1) * DH, :], in_=o_sb[:, :]
                    )

    nc.compile()
    return nc


def _get_nc(spads):
    key = tuple(spads)
    if key not in _CACHE:
        _CACHE[key] = build_nc(key)
    return _CACHE[key]


def _prepare(inputs):
    """Mask compaction, GBN folding, size-aware batch pairing, sharding."""
    import ml_dtypes

    q = np.asarray(inputs["q"], dtype=np.float32)
    k_in = np.asarray(inputs["k_in"], dtype=np.float32)
    v_in = np.asarray(inputs["v_in"], dtype=np.float32)
    k_w = np.asarray(inputs["k_w"], dtype=np.float32)
    v_w = np.asarray(inputs["v_w"], dtype=np.float32)
    v_b = np.asarray(inputs["v_b"], dtype=np.float32)
    gamma = np.asarray(inputs["gbn_gamma"], dtype=np.float32)
    gs = np.asarray(inputs["gbn_s"], dtype=np.float32)
    mask = np.asarray(inputs["mask"]).reshape(BS, SL)

    # GBN affine: only gamma/sd matters (additive part and k_b are softmax
    # shift-invariant); fold the scale into q per head.
    a = (gamma / gs).astype(np.float32)
    q_scaled = (
        (q.reshape(BS, H, DH, SL) * a[None, :, None, None]).reshape(BS, DA, SL)
    ).astype(np.float32)

    keeps = [np.flatnonzero(mask[b] == 0) for b in range(BS)]
    counts = np.array([len(k) for k in keeps])
    order = np.argsort(counts, kind="stable")
    # large batches first: the kernel tail (last head's epilogue chain) then
    # belongs to the cheaper small batch.
    slots = [order[N_CORES:], order[:N_CORES]]
    spads = tuple(
        _bucket(int(counts[sl].max()) if len(sl) else 128) for sl in slots
    )

    k_wT = np.ascontiguousarray(k_w.T).astype(np.float16)
    v_wT = np.ascontiguousarray(v_w.T).astype(ml_dtypes.bfloat16)
    vb2 = v_b.reshape(1, DA).astype(np.float32)

    in_maps = [
        {"kwT": k_wT, "vwT": v_wT, "vb": vb2} for _ in range(N_CORES)
    ]
    assign = np.zeros((N_CORES, B), dtype=np.int64)
    for s, (sl, spad) in enumerate(zip(slots, spads)):
        nch = len(_chunks(spad))
        for c in range(N_CORES):
            gb = int(sl[c])
            assign[c, s] = gb
            kidx = keeps[gb]
            n = len(kidx)
            qc = np.zeros((DA, spad), np.float16)
            kc = np.zeros((DA, spad), np.float16)
            vc = np.zeros((DA, spad), ml_dtypes.bfloat16)
            mf = np.zeros((nch * P,), np.float32)
            qc[:, :n] = q_scaled[gb][:, kidx].astype(np.float16)
            kc[:, :n] = k_in[gb][:, kidx].astype(np.float16)
            vc[:, :n] = v_in[gb][:, kidx].astype(ml_dtypes.bfloat16)
            mf[:n] = 1.0
            in_maps[c][f"q{s}"] = qc
            in_maps[c][f"kin{s}"] = kc
            in_maps[c][f"vin{s}"] = vc
            in_maps[c][f"mf{s}"] = mf
    return in_maps, keeps, assign, spads


def _scatter(results, keeps, assign) -> np.ndarray:
    out = np.zeros((BS, DA, SL), np.float32)
    for c in range(N_CORES):
        for s in range(B):
            gb = int(assign[c, s])
            kidx = keeps[gb]
            oc = np.asarray(results[c][f"out{s}"]).astype(np.float32)
            out[gb][:, kidx] = oc[:, : len(kidx)]
    return out


def kernel(**inputs) -> np.ndarray:
    from concourse.bass_utils import run_bass_kernel_spmd

    in_maps, keeps, assign, spads = _prepare(inputs)
    nc = _get_nc(spads)
    res = run_bass_kernel_spmd(nc, in_maps, list(range(N_CORES)))
    return _scatter(res.results, keeps, assign)
